# revision 11
# baseline (speedup 1.0000x reference)
"""Trainium2 kernel for nn_ChatModel_29618094473399.

Strategy (matches sharding_hint): data-parallel over batch across 8 cores for
the dominant compute — the decoder logits GEMM [B*Td,160]@[160,32000] (~42
GFLOP, >95% of model FLOPs). The tiny sequential trunk (embeddings, 2 mamba
blocks, pooled heads, 128-step GRU — ~0.6 GFLOP total, latency-bound serial
scans) runs on host in fp32 numpy. Each core computes logits for its 4
sequences: lhsT = dec_out^T with a ones-row appended (K=161) so the vocab bias
rides in the weight matrix; rhs = [dec_out_w^T; dec_out_b] streamed in vocab
chunks. Matmuls run as float32r (full-rate fp32 PE path, N=500>=256).
"""

import numpy as np

EMBED = 128
STATE = 64
HID = 160
VOCAB = 32000
B = 32
T = 512
TD = 128
NCORES = 8
BPC = B // NCORES  # 4 sequences per core
ROWS = BPC * TD    # 512 rows per core
K = HID + 1        # 161: contraction dim with bias row
CH = 4000          # vocab chunk streamed to SBUF
NSUB = 500         # matmul free dim (<=512 fp32 PSUM bank)


def _ln(x, g, b, eps=1e-5):
    mu = x.mean(-1, keepdims=True)
    var = ((x - mu) ** 2).mean(-1, keepdims=True)
    return ((x - mu) / np.sqrt(var + eps) * g + b).astype(np.float32)


def _sigmoid(x):
    return (1.0 / (1.0 + np.exp(-x))).astype(np.float32)


def _mamba(x, norm_g, norm_b, in_w, in_b, conv_w, conv_b, dt_w, dt_b,
           b_w, b_b, c_w, c_b, a_log, d, out_w, out_b):
    res = x
    xn = _ln(x, norm_g, norm_b)
    proj = xn @ in_w.T + in_b
    x_proj, z = proj[..., :EMBED], proj[..., EMBED:]
    xp = np.pad(x_proj, ((0, 0), (1, 1), (0, 0)))
    xc = (xp[:, :-2] * conv_w[:, 0] + xp[:, 1:-1] * conv_w[:, 1]
          + xp[:, 2:] * conv_w[:, 2] + conv_b)
    x_conv = np.tanh(xc).astype(np.float32)
    dt = np.logaddexp(0.0, z @ dt_w.T + dt_b).astype(np.float32) + 1e-4
    bt = np.tanh(z @ b_w.T + b_b).astype(np.float32)
    ct = np.tanh(z @ c_w.T + c_b).astype(np.float32)
    a = -np.exp(a_log)
    decay = np.exp(a * dt).astype(np.float32)
    states = np.empty_like(bt)
    s = np.zeros((x.shape[0], STATE), np.float32)
    for t in range(x.shape[1]):
        s = s * decay[:, t] + bt[:, t]
        states[:, t] = s
    y = ct * states
    mix = np.concatenate([x_conv * d, y], axis=-1)
    return (res + mix @ out_w.T + out_b).astype(np.float32)


def _trunk(x, lengths, resp_in, emb_table, m_norm_g, m_norm_b, m_in_w, m_in_b,
           m_conv_w, m_conv_b, m_dt_w, m_dt_b, m_b_w, m_b_b, m_c_w, m_c_b,
           m_a_log, m_d, m_out_w, m_out_b, pool_g, pool_b, shared_w, shared_b,
           intent_w, intent_b, style_w, style_b, cap_w, cap_b, op_w, op_b,
           dec_emb, dec_init_w, dec_init_b, gru_w_ih, gru_w_hh, gru_b_ih,
           gru_b_hh, dec_out_w, dec_out_b):
    h_seq = emb_table[x].astype(np.float32)
    for i in range(2):
        h_seq = _mamba(h_seq, m_norm_g[i], m_norm_b[i], m_in_w[i], m_in_b[i],
                       m_conv_w[i], m_conv_b[i], m_dt_w[i], m_dt_b[i],
                       m_b_w[i], m_b_b[i], m_c_w[i], m_c_b[i], m_a_log[i],
                       m_d[i], m_out_w[i], m_out_b[i])
    mask = (np.arange(T)[None, :] < lengths[:, None])[..., None]
    denom = np.maximum(lengths, 1)[:, None].astype(np.float32)
    pooled = (h_seq * mask).sum(axis=1) / denom
    h = _ln(pooled, pool_g, pool_b)
    h = np.maximum(h @ shared_w.T + shared_b, 0.0).astype(np.float32)
    intent = (h @ intent_w.T + intent_b).astype(np.float32)
    style = (h @ style_w.T + style_b).astype(np.float32)
    cap = (h @ cap_w.T + cap_b).astype(np.float32)
    op = (h @ op_w.T + op_b).astype(np.float32)
    dec_in = dec_emb[resp_in].astype(np.float32)
    hh = np.tanh(h @ dec_init_w.T + dec_init_b).astype(np.float32)
    gx = (dec_in @ gru_w_ih.T + gru_b_ih).astype(np.float32)
    dec_out = np.empty((B, TD, HID), np.float32)
    for t in range(TD):
        gh = hh @ gru_w_hh.T + gru_b_hh
        xr, xz, xn_ = np.split(gx[:, t], 3, axis=-1)
        hr, hz, hn = np.split(gh, 3, axis=-1)
        r = _sigmoid(xr + hr)
        zg = _sigmoid(xz + hz)
        n = np.tanh(xn_ + r * hn).astype(np.float32)
        hh = ((1.0 - zg) * n + zg * hh).astype(np.float32)
        dec_out[:, t] = hh
    return intent, style, cap, op, h, dec_out


_NC_CACHE = {}


def _build_bass():
    import concourse.mybir as mybir
    import concourse.tile as tile
    from concourse import bacc

    nc = bacc.Bacc("TRN2", target_bir_lowering=False)
    f32 = mybir.dt.float32
    f32r = mybir.dt.float32r
    bf16 = mybir.dt.bfloat16
    dec_t = nc.dram_tensor("dec_t", [K, ROWS], bf16, kind="ExternalInput")
    wt = nc.dram_tensor("wt", [K, VOCAB], bf16, kind="ExternalInput")
    out = nc.dram_tensor("logits", [ROWS, VOCAB], f32, kind="ExternalOutput")

    with tile.TileContext(nc) as tc:
        with (
            tc.tile_pool(name="lhs", bufs=1) as lhs_pool,
            tc.tile_pool(name="w", bufs=3) as w_pool,
            tc.tile_pool(name="ps", bufs=8, space="PSUM") as ps_pool,
            tc.tile_pool(name="stage", bufs=4) as stage_pool,
        ):
            decA = lhs_pool.tile([128, ROWS], bf16)
            decB = lhs_pool.tile([K - 128, ROWS], bf16)
            nc.gpsimd.dma_start(decA[:], dec_t[0:128, :])
            nc.gpsimd.dma_start(decB[:], dec_t[128:K, :])
            for c in range(VOCAB // CH):
                wA = w_pool.tile([128, CH], bf16, tag="wA")
                wB = w_pool.tile([K - 128, CH], bf16, tag="wB")
                nc.gpsimd.dma_start(wA[:], wt[0:128, c * CH:(c + 1) * CH])
                nc.gpsimd.dma_start(wB[:], wt[128:K, c * CH:(c + 1) * CH])
                for m in range(ROWS // 128):
                    st = stage_pool.tile([128, CH], f32, tag="stage")
                    for s in range(CH // NSUB):
                        ps = ps_pool.tile([128, NSUB], f32)
                        nc.tensor.matmul(
                            ps[:],
                            decA[:, m * 128:(m + 1) * 128],
                            wA[:, s * NSUB:(s + 1) * NSUB],
                            start=True, stop=False)
                        nc.tensor.matmul(
                            ps[:],
                            decB[:, m * 128:(m + 1) * 128],
                            wB[:, s * NSUB:(s + 1) * NSUB],
                            start=False, stop=True)
                        if m % 2 == 0:
                            nc.scalar.copy(st[:, s * NSUB:(s + 1) * NSUB], ps[:])
                        else:
                            nc.vector.tensor_copy(
                                st[:, s * NSUB:(s + 1) * NSUB], ps[:])
                    nc.gpsimd.dma_start(
                        out[m * 128:(m + 1) * 128, c * CH:(c + 1) * CH],
                        st[:])
    nc.compile()
    return nc


def kernel(**inputs):
    inputs = {k: np.asarray(v) for k, v in inputs.items()}
    fp = {k: (v.astype(np.float32) if v.dtype in (np.float64,) else v)
          for k, v in inputs.items()}
    intent, style, cap, op, h, dec_out = _trunk(**fp)

    dec_out_w = np.asarray(fp["dec_out_w"], np.float32)
    dec_out_b = np.asarray(fp["dec_out_b"], np.float32)
    wt = np.ascontiguousarray(
        np.concatenate([dec_out_w.T, dec_out_b[None, :]], axis=0),
        dtype=np.float32)  # [161, 32000]
    import ml_dtypes
    wt = wt.astype(ml_dtypes.bfloat16)

    in_maps = []
    for i in range(NCORES):
        d = dec_out[i * BPC:(i + 1) * BPC].reshape(ROWS, HID)
        import ml_dtypes
        dec_t = np.ascontiguousarray(
            np.concatenate([d.T, np.ones((1, ROWS), np.float32)],
                           axis=0)).astype(ml_dtypes.bfloat16)
        in_maps.append({"dec_t": dec_t, "wt": wt})

    from concourse.bass_utils import run_bass_kernel_spmd
    if "nc" not in _NC_CACHE:
        _NC_CACHE["nc"] = _build_bass()
    res = run_bass_kernel_spmd(_NC_CACHE["nc"], in_maps,
                               core_ids=list(range(NCORES)))
    resp_logits = np.concatenate(
        [r["logits"].reshape(BPC, TD, VOCAB) for r in res.results], axis=0)
    return (intent, style, cap, op, resp_logits, h)
